# revision 5
# baseline (speedup 1.0000x reference)
"""Trainium2 Bass kernel for nn_AttentionTIE (TIE-style edge-LayerNorm attention).

Sharding: 8 cores = (batch b = core//2) x (receiver-row half = core%2).
Each core computes the full v_sender for its batch, attention for its 1536
receiver rows, and the three projected outputs for those rows.

Design notes (cost-model predicted ~188us/core vs 412us baseline):
  - all big matmuls stream float32r: 1 cyc/row at free>=256 vs 4 for f32.
    FP32r needs rounded producers, so every matmul input lives in an
    F32R-typed tile written by DMA/ACT/DVE (bitcasts only at non-matmul
    read sites)
  - sender values centered in place (c_s = v_s - m_s): the q.c_s and
    cross matmuls absorb the m_s rank-1 terms, killing two aug passes;
    m_s itself comes from host-precomputed rank-1 vectors applied to the
    raw input streams so centering never waits on the full v_s chain
  - T = 1/std computed as Exp(-0.5*Ln(std2)): Ln and Exp share one ACT
    table ('natural_log_exp_and_others', pinned via a pre-placed
    InstLoadActFuncSet) so the main loop never reloads activation tables
    (the v1 baseline lost 124us to Sqrt<->Exp table thrash)
  - P pipeline in bf16 (tcc/pc/ptc/PT^T), 1024-wide ACT/DVE ops over
    pairs of PSUM banks (matmuls still write 512-wide, one bank each)
  - emission interleaves receiver-chunk prep and per-i-tile epilogues/
    projections into the ACT-saturated attention loop; masks prefetched
    one i-tile ahead (u8 DMA + Pool convert to -60*bf16)

Algorithm per core (all shapes [partition, free]):
  v_sT  = W_send @ xT + W_mem @ sendT + res_sT            [C, N]
  c_sT  = v_sT - 1 (x) m_s                                [C, N]
  v_rT  = W_recv @ xT_own + W_mem @ recvT_own + res_rT    [C, No]
  qT    = (W_q*scale) @ xT_own                            [C, No]
  std2[i,j] = (u_i+eps) + w_j + (2/D) v_r . c_s           (w_j via rank-1 aug)
  score[i,j] = q . c_s - M*mask ; uc = (score+alpha_i)*T
  T = Exp(-0.5 Ln(std2+bias)); P = Exp(uc) (row denom via ACT accum)
  PT = P*T; PT^T via PE transpose (bf16); pv = PT @ [v_s | 1 | m_s]
  out = (pv + A*v_r - (m_r A + MS)) / denom
  outputs: W'_proj @ out^T + b', W_r @ v_rT + r_b, W_s @ v_sT_own + s_b
"""
import os
import sys
from contextlib import ExitStack

import numpy as np

sys.path.insert(0, "/opt/trn_rl_repo")

import ml_dtypes  # noqa: E402
import concourse.bass as bass  # noqa: E402
import concourse.tile as tile  # noqa: E402
from concourse import bacc  # noqa: E402
from concourse import mybir  # noqa: E402
from concourse.bass_utils import run_bass_kernel_spmd  # noqa: E402

N, B, C = 3072, 4, 128
NO = N // 2          # own receiver rows per core
ITI = NO // 128      # 12 i-tiles
JCH = N // 512       # 6 j-chunks
JT = N // 128        # 24 j-tiles
EPS = 1e-5
SCALE = C ** -0.5
MASKM = 60.0         # masked-score bias: exp((score-M)*T) <= ~1e-14, ACT-range safe

F32 = mybir.dt.float32
F32R = mybir.dt.float32r
BF16 = mybir.dt.bfloat16
U8 = mybir.dt.uint8
AF = mybir.ActivationFunctionType
ALU = mybir.AluOpType
AX = mybir.AxisListType

_CACHE = {}


def _build_program():
    nc = bacc.Bacc("TRN2", target_bir_lowering=False, debug=False, num_devices=8)

    def din(name, shape, dtype=F32):
        return nc.dram_tensor(name, list(shape), dtype, kind="ExternalInput").ap()

    def dout(name, shape, dtype=F32):
        return nc.dram_tensor(name, list(shape), dtype, kind="ExternalOutput").ap()

    xT_d = din("xT", [C, N])
    xTo_d = din("xTo", [C, NO])
    sendT_d = din("sendT", [C, N])
    sendTo_d = din("sendTo", [C, NO])
    res_sT_d = din("res_sT", [C, N])
    res_sTo_d = din("res_sTo", [C, NO])
    recvTo_d = din("recvTo", [C, NO])
    res_rTo_d = din("res_rTo", [C, NO])
    mask_d = din("mask", [NO, N], U8)
    # all [C,*] f32 constants batched into one DMA: 8 CxC blocks + 5 columns
    wpack_d = din("wpack", [C, 8 * C + 7])
    idb_d = din("idb", [C, C], BF16)
    ones_row_d = din("ones_row", [1, NO])

    scr_mr_d = nc.dram_tensor("scr_mr", [1, NO], F32).ap()
    scr_ue_d = nc.dram_tensor("scr_ue", [1, NO], F32).ap()
    scr_al_d = nc.dram_tensor("scr_al", [1, NO], F32).ap()
    scr_ms_d = nc.dram_tensor("scr_ms", [1, N], F32).ap()
    outT_d = dout("outT", [C, NO])
    vr2T_d = dout("vr2T", [C, NO])
    vs2T_d = dout("vs2T", [C, NO])

    def r32(ap):
        return ap.bitcast(F32R)

    def f32v(ap):
        return ap.bitcast(F32)

    with tile.TileContext(nc) as tc, ExitStack() as ctx:
        const = ctx.enter_context(tc.tile_pool(name="const", bufs=1))
        per = ctx.enter_context(tc.tile_pool(name="per", bufs=1))
        stat = ctx.enter_context(tc.tile_pool(name="stat", bufs=1))
        stmp = ctx.enter_context(tc.tile_pool(name="stmp", bufs=2))
        ck = ctx.enter_context(tc.tile_pool(name="ck", bufs=2))
        strm = ctx.enter_context(tc.tile_pool(name="strm", bufs=8))
        mpool = ctx.enter_context(tc.tile_pool(name="mask", bufs=2))
        ps_mm = ctx.enter_context(tc.tile_pool(name="ps_mm", bufs=2, space="PSUM"))
        ps_tp = ctx.enter_context(tc.tile_pool(name="ps_tp", bufs=2, space="PSUM"))
        ps_pv = ctx.enter_context(tc.tile_pool(name="ps_pv", bufs=1, space="PSUM"))

        # Pre-place a load of the 'natural_log_exp_and_others' ACT table: it
        # serves every activation this kernel uses (Ln, Exp, Square, Copy,
        # Identity), so the compile-time table-load pass inserts no further
        # loads (the greedy default would thrash Ln<->Exp tables every chunk).
        nc.scalar.add_instruction(mybir.InstLoadActFuncSet(
            name=nc.get_next_instruction_name(), engine=mybir.EngineType.Activation,
            act_func_set_id=6, ins=[], outs=[]))

        # ---------------- constants ----------------
        def cload(name, d_ap, shape, dtype=F32):
            t = const.tile(shape, dtype, tag=name, name=name)
            nc.sync.dma_start(t[:], d_ap)
            return t

        wpack = const.tile([C, 8 * C + 7], F32R, tag="wpack", name="wpack")
        nc.sync.dma_start(wpack[:], wpack_d.bitcast(F32R))
        w_send = wpack[:, 0 * C:1 * C]
        w_mem = wpack[:, 1 * C:2 * C]
        w_recv = wpack[:, 2 * C:3 * C]
        w_qs = wpack[:, 3 * C:4 * C]
        w_proj = wpack[:, 4 * C:5 * C]
        w_r = wpack[:, 5 * C:6 * C]
        w_s = wpack[:, 6 * C:7 * C]
        idf = wpack[:, 7 * C:8 * C]
        bp = f32v(wpack[:, 8 * C + 0:8 * C + 1])
        br_c = f32v(wpack[:, 8 * C + 1:8 * C + 2])
        bs_c = f32v(wpack[:, 8 * C + 2:8 * C + 3])
        oneD = wpack[:, 8 * C + 3:8 * C + 4]
        one = wpack[:, 8 * C + 4:8 * C + 5]
        u1ms = wpack[:, 8 * C + 5:8 * C + 6]
        u2ms = wpack[:, 8 * C + 6:8 * C + 7]
        idb = cload("idb", idb_d, [C, C], BF16)
        ones_row = const.tile([1, NO], F32R)
        nc.sync.dma_start(ones_row[:], ones_row_d.bitcast(F32R))

        # persistent tensors
        v_sT = per.tile([C, N], F32R)
        c_sT = v_sT  # centered in place after sender stats are taken
        v_sTo = per.tile([C, NO], F32R)
        v_rT = per.tile([C, NO], F32R)
        qT = per.tile([C, NO], F32R)
        vr_s = per.tile([C, NO], F32R)         # v_rT * 2/C (cross lhsT)
        v_r_nat = per.tile([C, ITI * C], F32)
        v_s_bf = per.tile([C, N], BF16)
        v_s_aug = per.tile([C, JT * (C + 2)], BF16)
        outT_pre = per.tile([C, NO], F32R)

        m_r_row = stat.tile([1, NO], F32)
        u_eps_row = stat.tile([1, NO], F32)
        w_row = stat.tile([1, N], F32R)        # var_s per sender (rank-1 aug rhs)
        m_s_row = stat.tile([1, N], F32R)
        alpha_row = stat.tile([1, NO], F32)
        sumq_row = stat.tile([1, NO], F32)
        m_r_cols = stat.tile([C, ITI], F32)
        u_eps_cols = stat.tile([C, ITI], F32)
        al_cols = stat.tile([C, ITI], F32)
        m_s_cols = stat.tile([C, JT], F32)

        def stream(d_ap, sl, dt=F32, eng=None):
            t = strm.tile([C, 512], dt, tag="instream", name="instream")
            src = d_ap[:, sl]
            (eng or nc.sync).dma_start(t[:], src.bitcast(dt) if dt is F32R else src)
            return t

        v_s_aug_r = v_s_aug[:].rearrange("p (t c) -> p t c", c=C + 2)
        nc.gpsimd.memset(v_s_aug_r[:, :, C:C + 1], 1.0)

        # prefetch the first i-tile's mask so its DMA + Pool convert clear
        # before the attention loop starts
        mask_tiles = {}
        for it in (0,):
            mk8 = mpool.tile([C, N], U8, tag="mk8", name="mk8")
            nc.sync.dma_start(mk8[:], mask_d[bass.ts(it, 128), :])
            mkb = mpool.tile([C, N], BF16, tag="mkb", name="mkb")
            nc.gpsimd.tensor_scalar_mul(mkb[:], mk8[:], -MASKM)
            mask_tiles[it] = mkb

        # -------- phase 1: value tensors (inputs streamed chunk-wise) --------
        # order: sender chunks 0-1, receiver chunk 0 (gates the attention
        # loop start), then sender chunks 2-5
        def emit_sender_chunk(jc):
            sl = bass.ts(jc, 512)
            xc = stream(xT_d, sl, F32R)
            sc = stream(sendT_d, sl, F32R)
            rc = stream(res_sT_d, sl, F32R)
            ps = ps_mm.tile([C, 512], F32, tag="mm", name="mm")
            nc.tensor.matmul(ps[:], w_send, xc[:], start=True, stop=False)
            nc.tensor.matmul(ps[:], w_mem, sc[:], start=False, stop=True)
            nc.vector.tensor_tensor(out=v_sT[:, sl], in0=ps[:], in1=f32v(rc[:]), op=ALU.add)
            # m_s directly from the streams (host-precomputed rank-1 vectors),
            # so centering does not wait for the full v_sT chain
            psm = ps_pv.tile([1, 512], F32, tag="row", name="row")[:]
            nc.tensor.matmul(psm, u1ms, xc[:], start=True, stop=False)
            nc.tensor.matmul(psm, u2ms, sc[:], start=False, stop=False)
            nc.tensor.matmul(psm, oneD, rc[:], start=False, stop=True)
            nc.vector.tensor_copy(m_s_row[:, sl], psm)
            sqc = ck.tile([C, 512], F32R, tag="sqc", name="sqc")
            nc.scalar.activation(sqc[:], f32v(v_sT[:, sl]), AF.Square)
            psq = ps_pv.tile([1, 512], F32, tag="row", name="row")[:]
            nc.tensor.matmul(psq, oneD, sqc[:], start=True, stop=True)
            trow = stmp.tile([1, 512], F32, tag="trow", name="trow")
            nc.scalar.activation(trow[:], f32v(m_s_row[0:1, sl]), AF.Square)
            nc.vector.tensor_tensor(out=w_row[:, sl], in0=psq, in1=trow[:], op=ALU.subtract)
            nc.gpsimd.tensor_copy(v_s_bf[:, sl], f32v(v_sT[:, sl]))
            psb = ps_mm.tile([C, 512], F32, tag="mm", name="mm")
            nc.tensor.matmul(psb[:], ones_row[0:1, 0:C], m_s_row[0:1, sl], start=True, stop=True)
            nc.vector.tensor_tensor(out=c_sT[:, sl], in0=f32v(v_sT[:, sl]), in1=psb[:], op=ALU.subtract)
            # aug main blocks for this chunk (PE transposes; PE is idle here)
            pst = ps_tp.tile([C, 512], BF16, tag="tp", name="tp")
            for t in range(4):
                jt4 = jc * 4 + t
                nc.tensor.transpose(pst[:, bass.ts(t, 128)], v_s_bf[:, bass.ts(jt4, 128)], idb[:])
            src = pst[:].rearrange("p (t c) -> p t c", c=C)
            nc.scalar.copy(v_s_aug_r[:, jc * 4:(jc + 1) * 4, 0:C], src)

        # m_s col layout + aug m_s column (after all sender chunks)
        def emit_ms_cols():
            nc.sync.dma_start(scr_ms_d, f32v(m_s_row[:]))
            nc.sync.dma_start(m_s_cols[:], scr_ms_d.rearrange("o (t p) -> (o p) t", p=128))
            m_s_cols_r = m_s_cols[:].rearrange("p (t o) -> p t o", o=1)
            nc.scalar.copy(v_s_aug_r[:, :, C + 1:C + 2], m_s_cols_r)

        def emit_receiver_chunk(c3, stages=(0, 1, 2, 3)):
            sl = bass.ts(c3, 512)
            if 0 in stages:
                emit_receiver_s0(c3, sl)
            if 1 in stages:
                emit_receiver_s1(c3, sl)
            if 2 in stages:
                emit_receiver_s2(c3, sl)
            if 3 in stages:
                emit_receiver_s3(c3, sl)

        def emit_receiver_s0(c3, sl):
            xc = stream(xTo_d, sl, F32R)
            sc = stream(sendTo_d, sl, F32R)
            rc = stream(res_sTo_d, sl, F32)
            ps = ps_mm.tile([C, 512], F32, tag="mm", name="mm")
            nc.tensor.matmul(ps[:], w_send, xc[:], start=True, stop=False)
            nc.tensor.matmul(ps[:], w_mem, sc[:], start=False, stop=True)
            nc.vector.tensor_tensor(out=v_sTo[:, sl], in0=ps[:], in1=rc[:], op=ALU.add)
            rcv = stream(recvTo_d, sl, F32R)
            rrc = stream(res_rTo_d, sl, F32)
            ps2 = ps_mm.tile([C, 512], F32, tag="mm", name="mm")
            nc.tensor.matmul(ps2[:], w_recv, xc[:], start=True, stop=False)
            nc.tensor.matmul(ps2[:], w_mem, rcv[:], start=False, stop=True)
            nc.vector.tensor_tensor(out=v_rT[:, sl], in0=ps2[:], in1=rrc[:], op=ALU.add)
            ps3 = ps_mm.tile([C, 512], F32, tag="mm", name="mm")
            nc.tensor.matmul(ps3[:], w_qs, xc[:], start=True, stop=True)
            nc.vector.tensor_copy(qT[:, sl], ps3[:])

        def emit_receiver_s1(c3, sl):
            # receiver stats
            psm = ps_pv.tile([1, 512], F32, tag="row", name="row")[:]
            nc.tensor.matmul(psm, oneD, v_rT[:, sl], start=True, stop=True)
            nc.vector.tensor_copy(m_r_row[:, sl], psm)
            sqc = ck.tile([C, 512], F32R, tag="sqc", name="sqc")
            nc.scalar.activation(sqc[:], f32v(v_rT[:, sl]), AF.Square)
            psq = ps_pv.tile([1, 512], F32, tag="row", name="row")[:]
            nc.tensor.matmul(psq, oneD, sqc[:], start=True, stop=True)
            nc.vector.tensor_scalar_add(u_eps_row[:, sl], psq, EPS)
            trow = stmp.tile([1, 512], F32, tag="trow", name="trow")
            nc.scalar.activation(trow[:], m_r_row[:, sl], AF.Square)
            nc.gpsimd.tensor_tensor(out=u_eps_row[:, sl], in0=u_eps_row[:, sl], in1=trow[:], op=ALU.subtract)

        def emit_receiver_s2(c3, sl):
            pss = ps_pv.tile([1, 512], F32, tag="row", name="row")[:]
            nc.tensor.matmul(pss, one, qT[:, sl], start=True, stop=True)
            nc.vector.tensor_copy(sumq_row[:, sl], pss)
            qv = ck.tile([C, 512], F32R, tag="sqc", name="sqc")
            nc.vector.tensor_tensor(out=qv[:], in0=f32v(qT[:, sl]), in1=f32v(v_rT[:, sl]), op=ALU.mult)
            psa = ps_pv.tile([1, 512], F32, tag="row", name="row")[:]
            nc.tensor.matmul(psa, one, qv[:], start=True, stop=True)
            trow2 = stmp.tile([1, 512], F32, tag="trow", name="trow")
            nc.gpsimd.tensor_tensor(out=trow2[:], in0=sumq_row[0:1, sl], in1=m_r_row[:, sl], op=ALU.mult)
            nc.vector.tensor_tensor(out=alpha_row[:, sl], in0=psa, in1=trow2[:], op=ALU.subtract)
            # row -> col roundtrips for this chunk's four i-tiles
            csl = slice(c3 * 4, c3 * 4 + 4)
            nc.sync.dma_start(scr_mr_d[:, sl], m_r_row[:, sl])
            nc.sync.dma_start(m_r_cols[:, csl], scr_mr_d[:, sl].rearrange("o (t p) -> (o p) t", p=128))
            nc.sync.dma_start(scr_ue_d[:, sl], u_eps_row[:, sl])
            nc.sync.dma_start(u_eps_cols[:, csl], scr_ue_d[:, sl].rearrange("o (t p) -> (o p) t", p=128))
            nc.sync.dma_start(scr_al_d[:, sl], alpha_row[:, sl])
            nc.sync.dma_start(al_cols[:, csl], scr_al_d[:, sl].rearrange("o (t p) -> (o p) t", p=128))

        def emit_receiver_s3(c3, sl):
            # v_r natural + scaled v_r for this chunk
            pst = ps_tp.tile([C, 512], F32R, tag="tp", name="tp")
            for t in range(4):
                it = c3 * 4 + t
                nc.tensor.transpose(pst[:, bass.ts(t, 128)], v_rT[:, bass.ts(it, 128)], idf)
            nc.vector.tensor_copy(v_r_nat[:, bass.ts(c3, 512)], f32v(pst[:]))
            nc.vector.tensor_scalar_mul(vr_s[:, sl], f32v(v_rT[:, sl]), 2.0 / C)
            # r/s projections for this chunk (fills engine gaps later)
            for w, bias_col, rhs, out_d in (
                (w_r, br_c, v_rT, vr2T_d),
                (w_s, bs_c, v_sTo, vs2T_d),
            ):
                psj = ps_mm.tile([C, 512], F32, tag="mm", name="mm")
                nc.tensor.matmul(psj[:], w, rhs[:, sl], start=True, stop=True)
                ob = stmp.tile([C, 512], F32, tag="ob", name="ob")
                nc.scalar.activation(ob[:], psj[:], AF.Identity, bias=bias_col)
                nc.sync.dma_start(out_d[:, sl], ob[:])

        GRW = 1024
        NGR = N // GRW

        def emit_itile(it):
            isl = bass.ts(it, 128)
            mkb = mask_tiles.pop(it)
            nit = it + 1
            if nit < ITI:
                mk8 = mpool.tile([C, N], U8, tag="mk8", name="mk8")
                nc.sync.dma_start(mk8[:], mask_d[bass.ts(nit, 128), :])
                mkb2 = mpool.tile([C, N], BF16, tag="mkb", name="mkb")
                nc.gpsimd.tensor_scalar_mul(mkb2[:], mk8[:], -MASKM)
                mask_tiles[nit] = mkb2

            den_part = stmp.tile([C, 4], F32, tag="den_part", name="den_part")
            pv = ps_pv.tile([C, C + 2], F32, tag="pv", name="pv")

            for jc in range(NGR):
                jsl = bass.ts(jc, GRW)
                # std2 = (2/D) v_r.c_s  +  1 (x) w_j   (+ u_i + eps via Ln bias)
                # matmuls run as 512-wide halves (PSUM bank limit); the
                # ACT/DVE consumers read the full GRW-wide tile
                ps_v = ps_mm.tile([C, GRW], F32, tag="mm", name="mm")
                ps_s = ps_mm.tile([C, GRW], F32, tag="mm", name="mm")
                for h in range(GRW // 512):
                    hsl = bass.ts(2 * jc + h, 512)
                    hps = slice(512 * h, 512 * (h + 1))
                    nc.tensor.matmul(ps_v[:, hps], vr_s[:, isl], c_sT[:, hsl], start=True, stop=False)
                    nc.tensor.matmul(ps_v[:, hps], ones_row[0:1, isl], w_row[0:1, hsl], start=False, stop=True)
                    # score = q.c_s - M*mask   (alpha added in the stt below)
                    nc.tensor.matmul(ps_s[:, hps], qT[:, isl], c_sT[:, hsl], start=True, stop=False)
                    nc.tensor.matmul(ps_s[:, hps], idb[:], mkb[:, hsl], start=False, stop=True)

                # T = exp(-0.5 ln(std2)); Ln and Exp share one ACT table
                lc = ck.tile([C, GRW], F32, tag="lc", name="lc")
                nc.scalar.activation(lc[:], ps_v[:], AF.Ln, bias=u_eps_cols[:, it:it + 1])
                tcc = ck.tile([C, GRW], BF16, tag="tcc", name="tcc")
                nc.scalar.activation(tcc[:], lc[:], AF.Exp, scale=-0.5)
                uc = ck.tile([C, GRW], F32, tag="uc", name="uc")
                nc.vector.scalar_tensor_tensor(
                    out=uc[:], in0=ps_s[:], scalar=al_cols[:, it:it + 1], in1=tcc[:],
                    op0=ALU.add, op1=ALU.mult)
                pc = ck.tile([C, GRW], BF16, tag="pc", name="pc")
                nc.scalar.activation(pc[:], uc[:], AF.Exp, accum_out=den_part[:, jc:jc + 1])
                ptc = ck.tile([C, GRW], BF16, tag="ptc", name="ptc")
                nc.vector.tensor_tensor(out=ptc[:], in0=pc[:], in1=tcc[:], op=ALU.mult)

                pst = ps_tp.tile([C, GRW], BF16, tag="tp", name="tp")
                for t in range(GRW // 128):
                    nc.tensor.transpose(pst[:, bass.ts(t, 128)], ptc[:, bass.ts(t, 128)], idb[:])
                pttc = ck.tile([C, GRW], BF16, tag="pttc", name="pttc")
                nc.vector.tensor_copy(pttc[:], pst[:])
                for t in range(GRW // 128):
                    jt = jc * (GRW // 128) + t
                    nc.tensor.matmul(
                        pv[:], pttc[:, bass.ts(t, 128)], v_s_aug_r[:, jt, :],
                        start=(jc == 0 and t == 0), stop=(jc == NGR - 1 and t == GRW // 128 - 1))

            den = stmp.tile([C, 1], F32, tag="den", name="den")
            nc.vector.tensor_reduce(den[:], den_part[:, 0:NGR], axis=AX.X, op=ALU.add)
            rcol = stmp.tile([C, 1], F32, tag="rcol", name="rcol")
            nc.vector.reciprocal(rcol[:], den[:])
            ams = stmp.tile([C, 2], F32, tag="ams", name="ams")
            nc.vector.tensor_copy(ams[:], pv[:, C:C + 2])
            t1 = stmp.tile([C, 1], F32, tag="t1", name="t1")
            nc.vector.scalar_tensor_tensor(
                out=t1[:], in0=ams[:, 0:1], scalar=m_r_cols[:, it:it + 1], in1=ams[:, 1:2],
                op0=ALU.mult, op1=ALU.add)
            x1 = stmp.tile([C, C], F32, tag="x1", name="x1")
            nc.vector.scalar_tensor_tensor(
                out=x1[:], in0=v_r_nat[:, isl], scalar=ams[:, 0:1], in1=pv[:, 0:C],
                op0=ALU.mult, op1=ALU.add)
            # out_pre = (x1 - t1) * rcol
            x2 = stmp.tile([C, C], F32R, tag="x2", name="x2")
            nc.vector.tensor_scalar(
                out=x2[:], in0=x1[:], scalar1=t1[:, 0:1], scalar2=rcol[:, 0:1],
                op0=ALU.subtract, op1=ALU.mult)
            pso = ps_tp.tile([C, C], F32R, tag="tp", name="tp")
            nc.tensor.transpose(pso[:], x2[:], idf)
            nc.vector.tensor_copy(outT_pre[:, isl], f32v(pso[:]).bitcast(F32R))
            # final projection for this i-tile (overlaps with later i-tiles)
            pspj = ps_pv.tile([C, C], F32, tag="pv", name="pv")
            nc.tensor.matmul(pspj[:], w_proj, outT_pre[:, isl], start=True, stop=True)
            obj = stmp.tile([C, C], F32, tag="obj", name="obj")
            nc.vector.tensor_scalar_add(obj[:], pspj[:], bp)
            nc.sync.dma_start(outT_d[:, isl], obj[:])

        # emission order: sender chunks 0-1, then receiver chunk 0 (it gates
        # the attention-loop start; its DMAs jump the queue), then the rest
        emit_sender_chunk(0)
        emit_sender_chunk(1)
        emit_receiver_chunk(0)
        for jc in range(2, JCH):
            emit_sender_chunk(jc)
        emit_ms_cols()


        # interleave: stage chunk c3+1's prep one stage per i-tile of group c3
        for c3 in range(NO // 512):
            for k, it in enumerate(range(4 * c3, 4 * c3 + 4)):
                emit_itile(it)
                if c3 + 1 < NO // 512:
                    emit_receiver_chunk(c3 + 1, stages=(k,))

    nc.compile()
    return nc


def _host_prep(inputs):
    """Returns list of 8 per-core input dicts."""
    f32 = np.float32
    x = np.ascontiguousarray(np.asarray(inputs["x"], f32))
    recv = np.asarray(inputs["receiver_val_res"], f32)
    send = np.asarray(inputs["sender_val_res"], f32)
    res_r = np.asarray(inputs["residual_receiver"], f32)
    res_s = np.asarray(inputs["residual_sender"], f32)
    mask = np.asarray(inputs["attn_mask"])
    ra = np.asarray(inputs["relation_attn"], f32)
    q_w = np.asarray(inputs["q_w"], f32)
    proj_w = np.asarray(inputs["proj_w"], f32)
    proj_b = np.asarray(inputs["proj_b"], f32)
    r_w = np.asarray(inputs["r_w"], f32)
    r_b = np.asarray(inputs["r_b"], f32)
    s_w = np.asarray(inputs["s_w"], f32)
    s_b = np.asarray(inputs["s_b"], f32)
    n_weight = np.asarray(inputs["n_weight"], f32)
    n_bias = np.asarray(inputs["n_bias"], f32)

    mem_w, recv_w, send_w = ra[:, :C], ra[:, C:2 * C], ra[:, 2 * C:]
    w_proj_eff = proj_w * n_weight[None, :]
    b_proj_eff = proj_w @ n_bias + proj_b

    cc = np.ascontiguousarray
    oneD_vec = np.full((C,), 1.0 / C, f32)
    wpack = np.concatenate([
        send_w.T, mem_w.T, recv_w.T, q_w.T * SCALE, w_proj_eff.T, r_w.T, s_w.T,
        np.eye(C, dtype=f32),
        b_proj_eff[:, None], r_b[:, None], s_b[:, None],
        np.full((C, 1), 1.0 / C, f32), np.ones((C, 1), f32),
        (send_w.T @ oneD_vec)[:, None], (mem_w.T @ oneD_vec)[:, None],
    ], axis=1).astype(f32)
    weights = {
        "wpack": cc(wpack),
        "idb": cc(np.eye(C).astype(ml_dtypes.bfloat16)),
        "ones_row": np.ones((1, NO), f32),
    }

    in_maps = []
    for core in range(8):
        b, half = core // 2, core % 2
        i0, i1 = half * NO, (half + 1) * NO
        xb = cc(x[:, b, :].T)                      # [C, N]
        sb = cc(send[:, b, :].T)
        rsb = cc(res_s[:, b, :].T)
        m = {
            "xT": xb, "xTo": cc(xb[:, i0:i1]),
            "sendT": sb, "sendTo": cc(sb[:, i0:i1]),
            "res_sT": rsb, "res_sTo": cc(rsb[:, i0:i1]),
            "recvTo": cc(recv[i0:i1, b, :].T),
            "res_rTo": cc(res_r[i0:i1, b, :].T),
            "mask": cc(mask[b, 0, i0:i1, :].astype(np.uint8)),
        }
        m.update(weights)
        in_maps.append(m)
    return in_maps


def kernel(**inputs):
    if "nc" not in _CACHE:
        _CACHE["nc"] = _build_program()
    nc = _CACHE["nc"]
    in_maps = _host_prep(inputs)
    res = run_bass_kernel_spmd(nc, in_maps, core_ids=list(range(8)))
    out = np.zeros((N, B, C), np.float32)
    vr2 = np.zeros((N, B, C), np.float32)
    vs2 = np.zeros((N, B, C), np.float32)
    for core in range(8):
        b, half = core // 2, core % 2
        i0, i1 = half * NO, (half + 1) * NO
        r = res.results[core]
        out[i0:i1, b, :] = r["outT"].T
        vr2[i0:i1, b, :] = r["vr2T"].T
        vs2[i0:i1, b, :] = r["vs2T"].T
    return out, vr2, vs2
